# revision 2
# baseline (speedup 1.0000x reference)
"""EnVAE sampling kernel v2 for 8x TRN2 NeuronCores.

Math (per group g, batch element b):
  Xg = X[:, g::8]                                      # (b, 128)
  h  = relu(Xg @ W1[g] + b1[g])                        # (b, 128)
  out= h @ W2[g] + b2[g]; means=out[:, :64]; lv=out[:, 64:]
  z  = means[b, idx] + eps * exp(0.5 * lv[b, idx])

Device (batch-sharded 8 ways):
  mm1  fp8e4 DoubleRow:  hp = W1g^T Xg         (PE, 0.5 cyc/row)
  relu ACT/GPS:          h = relu(hp + b1)     -> fp16
  mm2  fp16 combined:    bank = W2g^T h        # [128 = 64 mean | 64 logvar, b]
  had  DVE/GPS (pair-wide): prod = bank * ohdup
  zred PE:               z[b, 0] = sum_top64 prod, z[b, 1] = sum_bot64 prod
Host finishes: z = zM + b2m[idx] + eps * exp(0.5*(zL + b2v[idx]))
"""

import numpy as np
import ml_dtypes

import concourse.bass as bass
import concourse.bacc as bacc
import concourse.mybir as mybir
from concourse import tile
from concourse import bass_utils

OBS = 1024
LAT = 64
G = 8
GS = 128
HID = 128
BATCH = 65536
NCORES = 8
BPC = BATCH // NCORES        # 8192 batch rows per core
SC = 512                     # batch rows per chunk instance
NPAIR = G // 2
QUAD = 4                     # sc-chunks per quad (DMA granule)
QW = QUAD * SC               # 2048
F16 = mybir.dt.float16
F32 = mybir.dt.float32
FP8 = mybir.dt.float8e4
NP_FP8 = ml_dtypes.float8_e4m3fn

GROUP_IDX = np.stack([np.arange(n, OBS, G) for n in range(G)])  # (g, gs)


def _mk_pattern(total, n_gps):
    pat = ['x'] * total
    if n_gps > 0:
        for k in range(n_gps):
            pat[(k * total) // n_gps] = 'G'
    return pat


def build_program(nsc: int, num_devices: int = NCORES, relu_dve=0, debug=False):
    """Per-core program for nsc chunks of SC batch rows."""
    B = nsc * SC
    nquad = nsc // QUAD
    NM = (nsc // 2) * G                 # 64 wide-instances (g, 2*sc)
    NREL = NM * 2                       # 128 narrow relu ops
    nc = bacc.Bacc("TRN2", target_bir_lowering=False, debug=False,
                   num_devices=num_devices)

    relu_pat = ['x'] * NREL                  # 'x' -> ACT, 'D' -> DVE
    for k in range(relu_dve):
        relu_pat[(k * NREL) // relu_dve] = 'D'

    # DRAM inputs (per-core shard)
    xq = nc.dram_tensor("xq", [nquad, NPAIR, 128, 2, QW], FP8,
                        kind="ExternalInput").ap()
    # onehot (dup'd to 128 rows), per group: [nquad, g, 128, QW]
    oh = nc.dram_tensor("oh", [nquad, G, 128, QW], mybir.dt.int8,
                        kind="ExternalInput").ap()
    w1 = nc.dram_tensor("w1", [128, NPAIR, 2, HID], FP8,
                        kind="ExternalInput").ap()
    w2 = nc.dram_tensor("w2", [G, GS, HID], F16, kind="ExternalInput").ap()
    b1 = nc.dram_tensor("b1", [G, GS], F32, kind="ExternalInput").ap()
    sel2 = nc.dram_tensor("sel2", [2, 128], F16, kind="ExternalInput").ap()
    # out (f32, straight from psum): wide-inst m = (quad, half, g);
    # col = m*16 + (j*4+c)*2 + {0:mean, 1:logvar}
    zout = nc.dram_tensor("z", [128, NM * 16], F32,
                          kind="ExternalOutput").ap()
    if debug:
        dbg_bank = nc.dram_tensor("dbg_bank", [128, 2, SC], F32,
                                  kind="ExternalOutput").ap()
        dbg_prod = nc.dram_tensor("dbg_prod", [128, 2, SC], F16,
                                  kind="ExternalOutput").ap()
        dbg_h = nc.dram_tensor("dbg_h", [128, SC], F16,
                               kind="ExternalOutput").ap()

    from contextlib import ExitStack
    with tile.TileContext(nc) as tc, ExitStack() as st:
        cp = st.enter_context(tc.tile_pool(name="const", bufs=1))
        # warm the activation table before the DMAs so the one-time
        # LoadActFuncSet doesn't gate the first real relu
        warm = cp.tile([128, 1], F16, tag="warm")
        nc.vector.memset(warm[:], 0.0)
        nc.scalar.activation(warm[:], warm[:],
                             mybir.ActivationFunctionType.Relu,
                             bias=0.0, scale=1.0)
        w1t = cp.tile([128, NPAIR, 2, HID], FP8, tag="w1")
        nc.sync.dma_start(w1t[:], w1)
        w1_sb = [w1t[:, p] for p in range(NPAIR)]
        b1_sb = cp.tile([GS, G], F32, tag="b1")
        w2_sb = cp.tile([GS, G, HID], F16, tag="w2")
        sel2_sb = cp.tile([128, 2], F16, tag="sel2")

        def _load_small_consts():
            # emitted after the first x/oh DMAs: b1 is needed by the first
            # relu (~7us in), w2 by the first mm2, sel2 by the first zred
            nc.sync.dma_start(b1_sb[:], b1.rearrange("g k -> k g"))
            nc.sync.dma_start(w2_sb[:], w2.rearrange("g k m -> k g m"))
            nc.sync.dma_start(sel2_sb[:], sel2.rearrange("f k -> k f"))

        xpool = st.enter_context(tc.tile_pool(name="xp", bufs=2))
        ohpool = st.enter_context(tc.tile_pool(name="ohp", bufs=2))
        hpsum = st.enter_context(tc.tile_pool(name="hps", bufs=3, space="PSUM"))
        hpool = st.enter_context(tc.tile_pool(name="hsb", bufs=6))
        bkpsum = st.enter_context(tc.tile_pool(name="bkps", bufs=2, space="PSUM"))
        ppool = st.enter_context(tc.tile_pool(name="prod", bufs=3))
        zpool = st.enter_context(tc.tile_pool(name="zp", bufs=1, space="PSUM"))
        zsbp = st.enter_context(tc.tile_pool(name="zsb", bufs=1))

        zt = zpool.tile([128, 512], F32, tag="z")   # 32 wide-insts per fill
        zsb = zsbp.tile([128, NM * 16], F32, tag="zstage")

        # wide-instance m = (quad, half, g): one group x 1024 batch rows.
        # narrow step n = 2m + j (j = sc-half within the wide instance).
        # software pipeline: mm1(n) | relu(n-2) | mm2(m-2) | had(m-3) | zred(m-4)
        hp_t, hsb_t, bank_t, prod_t = {}, {}, {}, {}
        xg_t, ohq_t = {}, {}
        NN = NM * 2

        def ninfo(n):
            m, j = divmod(n, 2)
            qh, g = divmod(m, G)
            quad, half = divmod(qh, 2)
            return m, j, quad, half, g

        def st_mm1(n):
            m, j, quad, half, g = ninfo(n)
            pair, i = divmod(g, 2)
            so = (half * 2 + j) * SC
            hp = hpsum.tile([HID, SC], F32, name="hp", tag="hpsum")
            nc.tensor.matmul(
                hp[:], w1_sb[pair][64 * i:64 * i + 64],
                xg_t[quad][pair][64 * i:64 * i + 64, :, so:so + SC],
                start=True, stop=True,
                perf_mode=mybir.MatmulPerfMode.DoubleRow,
                tile_position=(64 * i, 0))
            hp_t[n] = hp

        def st_relu(n):
            m, j, quad, half, g = ninfo(n)
            hp = hp_t.pop(n)
            hsb = hpool.tile([HID, SC], F16, name="hsb", tag="h")
            if relu_pat[n] == 'D':
                nc.vector.tensor_scalar(
                    hsb[:], hp[:], b1_sb[:, g:g + 1], 0.0,
                    mybir.AluOpType.add, mybir.AluOpType.max)
            else:
                nc.scalar.activation(
                    hsb[:], hp[:], mybir.ActivationFunctionType.Relu,
                    bias=b1_sb[:, g:g + 1], scale=1.0)
            if debug and n == 0:
                nc.sync.dma_start(dbg_h[:], hsb[:])
            hsb_t[n] = hsb

        def st_mm2(m):
            _, _, quad, half, g = ninfo(2 * m)
            bank = bkpsum.tile([128, 2, SC], F32, name="bank", tag="bank")
            for j in range(2):
                nc.tensor.matmul(bank[:, j], w2_sb[:, g],
                                 hsb_t.pop(2 * m + j)[:],
                                 start=True, stop=True)
            bank_t[m] = bank

        def st_had(m):
            _, _, quad, half, g = ninfo(2 * m)
            bank = bank_t.pop(m)
            prod = ppool.tile([128, 2, SC], F16, name="prod", tag="prod")
            oht = ohq_t[quad][g][:, half]            # [128, 2, SC]
            if debug and m == 0:
                bsb = ppool.tile([128, 2, SC], F32, name="bsb", tag="bdbg")
                nc.vector.tensor_copy(bsb[:], bank[:])
                nc.sync.dma_start(dbg_bank[:], bsb[:])
            nc.vector.tensor_tensor(
                prod[:], bank[:], oht, mybir.AluOpType.mult)
            if debug and m == 0:
                nc.sync.dma_start(dbg_prod[:], prod[:])
            prod_t[m] = prod

        def st_zred(m):
            prod = prod_t.pop(m)
            zoff = (m % 32) * 16
            for c8 in range(8):
                j, c = divmod(c8, 4)
                nc.tensor.matmul(
                    zt[:, zoff + 2 * c8: zoff + 2 * c8 + 2],
                    prod[:, j, 128 * c:128 * c + 128],
                    sel2_sb[:], start=True, stop=True,
                    skip_group_check=True)
            if m % 16 == 15:
                k = (m // 16) % 2
                lo_, hi_ = (m - 15) * 16, (m + 1) * 16
                nc.vector.tensor_copy(zsb[:, lo_:hi_],
                                      zt[:, k * 256:(k + 1) * 256])
                nc.sync.dma_start(zout[:, lo_:hi_], zsb[:, lo_:hi_])

        for n in range(NN + 8):
            if n < NN:
                quad = n // (NN // nquad)
                if n % (NN // nquad) == 0:
                    xg = [xpool.tile([128, 2, QW], FP8, name=f"xg{p}",
                                     tag=f"xg{p}") for p in range(NPAIR)]
                    ohq = [ohpool.tile([128, 2, 2, SC], mybir.dt.int8,
                                       name=f"oh{g}", tag=f"oh{g}")
                           for g in range(G)]
                    # interleave so the pipeline fills asap: x for pair p
                    # arrives just before its mm1s, oh soon after
                    for p in range(NPAIR):
                        nc.sync.dma_start(xg[p][:], xq[quad, p])
                        if quad == 0 and p == 0:
                            _load_small_consts()
                        nc.sync.dma_start(ohq[2 * p][:], oh[quad, 2 * p])
                        nc.sync.dma_start(ohq[2 * p + 1][:],
                                          oh[quad, 2 * p + 1])
                    xg_t[quad] = xg
                    ohq_t[quad] = ohq
                st_mm1(n)
            if 2 <= n < NN + 2:
                st_relu(n - 2)
            if n % 2 == 1:
                m = (n - 1) // 2
                if 2 <= m < NM + 2:
                    st_mm2(m - 2)
                if 3 <= m < NM + 3:
                    st_had(m - 3)
                if 4 <= m < NM + 4:
                    st_zred(m - 4)

    nc.compile()
    return nc


# ---------------------------------------------------------------- host side --

def _prep_host(X, eps, W1, b1, W2, b2, indices, nsc=BPC // SC, ncores=NCORES):
    B = nsc * SC
    nquad = nsc // QUAD
    Xp = np.ascontiguousarray(X[:, GROUP_IDX.reshape(-1)]).reshape(BATCH, G, GS)
    Xp8 = Xp.astype(NP_FP8)
    # W1 DoubleRow pack: (128, pair, 2, hid); partition i*64+p = group 2*pair+i
    # k-slot (p, t) = gs index t*64+p, matching the X pack
    W1r = W1.astype(NP_FP8).reshape(G, 2, 64, HID).transpose(0, 2, 1, 3)
    w1dr = np.ascontiguousarray(
        W1r.reshape(NPAIR, 128, 2, HID).transpose(1, 0, 2, 3))
    w2h = W2.astype(np.float16)
    b1f = b1.astype(np.float32)
    sel2 = np.zeros((2, 128), np.float16)
    sel2[0, :64] = 1.0
    sel2[1, 64:] = 1.0

    in_maps = []
    for core in range(ncores):
        lo = core * B
        Xc = Xp8[lo:lo + B]                          # (B, g, 128)
        Xt = Xc.reshape(B, G, 2, 64).transpose(1, 3, 2, 0)   # (g,64,2,B)
        xpair = Xt.reshape(NPAIR, 128, 2, B)
        xqc = np.ascontiguousarray(
            xpair.reshape(NPAIR, 128, 2, nquad, QW).transpose(3, 0, 1, 2, 4))
        idxc = indices[:, lo:lo + B]                 # (G, B)
        ohm = (idxc[:, None, :] == np.arange(LAT)[None, :, None])  # (G,64,B)
        ohdup = np.concatenate([ohm, ohm], axis=1).astype(np.int8)  # (G,128,B)
        # -> [nquad, g, 128, QW]
        ohc = np.ascontiguousarray(
            ohdup.reshape(G, 128, nquad, QW).transpose(2, 0, 1, 3))
        in_maps.append({
            "xq": xqc, "oh": ohc, "w1": w1dr, "w2": w2h, "b1": b1f,
            "sel2": sel2,
        })
    return in_maps


def _unscramble(zdev, nsc):
    """zdev: (128, nwide*16) f16 -> zM, zL each (G, nsc*SC) f32.

    wide-inst m = (quad, half, g); col = m*16 + (j*4+c)*2 + q;
    batch b = (quad*4 + half*2 + j)*512 + c*128 + p.
    """
    B = nsc * SC
    nquad = nsc // QUAD
    zr = zdev.astype(np.float32).reshape(128, nquad, 2, G, 2, 4, 2)
    # [p, quad, half, g, j, c, q] -> [g, quad, half, j, c, p]
    zM = zr[..., 0].transpose(3, 1, 2, 4, 5, 0).reshape(G, B)
    zL = zr[..., 1].transpose(3, 1, 2, 4, 5, 0).reshape(G, B)
    return zM, zL


def _finish_host(zM, zL, b2, eps_c, idxc):
    b2m_sel = np.take_along_axis(b2[:, :LAT], idxc, axis=1)
    b2v_sel = np.take_along_axis(b2[:, LAT:], idxc, axis=1)
    return zM + b2m_sel + eps_c * np.exp(0.5 * (zL + b2v_sel))


_NC_CACHE = {}


def kernel(X, eps, W1, b1, W2, b2, indices):
    nsc = BPC // SC
    key = (nsc, NCORES)
    if key not in _NC_CACHE:
        _NC_CACHE[key] = build_program(nsc, NCORES)
    nc = _NC_CACHE[key]
    in_maps = _prep_host(X, eps, W1, b1, W2, b2, indices)
    res = bass_utils.run_bass_kernel_spmd(nc, in_maps,
                                          core_ids=list(range(NCORES)))
    z = np.zeros((G, BATCH), np.float32)
    B = nsc * SC
    for core in range(NCORES):
        lo = core * B
        zM, zL = _unscramble(res.results[core]["z"], nsc)
        z[:, lo:lo + B] = _finish_host(zM, zL, b2, eps[:, lo:lo + B],
                                       indices[:, lo:lo + B])
    return z.astype(np.float32)


# revision 3
# speedup vs baseline: 1.0049x; 1.0049x over previous
"""EnVAE sampling kernel v2 for 8x TRN2 NeuronCores.

Math (per group g, batch element b):
  Xg = X[:, g::8]                                      # (b, 128)
  h  = relu(Xg @ W1[g] + b1[g])                        # (b, 128)
  out= h @ W2[g] + b2[g]; means=out[:, :64]; lv=out[:, 64:]
  z  = means[b, idx] + eps * exp(0.5 * lv[b, idx])

Device (batch-sharded 8 ways):
  mm1  fp8e4 DoubleRow:  hp = W1g^T Xg         (PE, 0.5 cyc/row)
  relu ACT/GPS:          h = relu(hp + b1)     -> fp16
  mm2  fp16 combined:    bank = W2g^T h        # [128 = 64 mean | 64 logvar, b]
  had  DVE/GPS (pair-wide): prod = bank * ohdup
  zred PE:               z[b, 0] = sum_top64 prod, z[b, 1] = sum_bot64 prod
Host finishes: z = zM + b2m[idx] + eps * exp(0.5*(zL + b2v[idx]))
"""

import numpy as np
import ml_dtypes

import concourse.bass as bass
import concourse.bacc as bacc
import concourse.mybir as mybir
from concourse import tile
from concourse import bass_utils

OBS = 1024
LAT = 64
G = 8
GS = 128
HID = 128
BATCH = 65536
NCORES = 8
BPC = BATCH // NCORES        # 8192 batch rows per core
SC = 512                     # batch rows per chunk instance
NPAIR = G // 2
QUAD = 4                     # sc-chunks per quad (DMA granule)
QW = QUAD * SC               # 2048
F16 = mybir.dt.float16
F32 = mybir.dt.float32
FP8 = mybir.dt.float8e4
NP_FP8 = ml_dtypes.float8_e4m3fn

GROUP_IDX = np.stack([np.arange(n, OBS, G) for n in range(G)])  # (g, gs)


def _mk_pattern(total, n_gps):
    pat = ['x'] * total
    if n_gps > 0:
        for k in range(n_gps):
            pat[(k * total) // n_gps] = 'G'
    return pat


def build_program(nsc: int, num_devices: int = NCORES, relu_dve=0, debug=False):
    """Per-core program for nsc chunks of SC batch rows."""
    B = nsc * SC
    nquad = nsc // QUAD
    NM = (nsc // 2) * G                 # 64 wide-instances (g, 2*sc)
    NREL = NM * 2                       # 128 narrow relu ops
    nc = bacc.Bacc("TRN2", target_bir_lowering=False, debug=False,
                   num_devices=num_devices)

    relu_pat = ['x'] * NREL                  # 'x' -> ACT, 'D' -> DVE
    for k in range(relu_dve):
        relu_pat[(k * NREL) // relu_dve] = 'D'

    # DRAM inputs (per-core shard)
    xq = nc.dram_tensor("xq", [nquad, NPAIR, 128, 2, QW], FP8,
                        kind="ExternalInput").ap()
    # onehot (dup'd to 128 rows), per group: [nquad, g, 128, QW]
    oh = nc.dram_tensor("oh", [nquad, G, 128, QW], mybir.dt.int8,
                        kind="ExternalInput").ap()
    w1 = nc.dram_tensor("w1", [128, NPAIR, 2, HID], FP8,
                        kind="ExternalInput").ap()
    w2 = nc.dram_tensor("w2", [G, GS, HID], F16, kind="ExternalInput").ap()
    b1 = nc.dram_tensor("b1", [G, GS], F32, kind="ExternalInput").ap()
    sel2 = nc.dram_tensor("sel2", [2, 128], F16, kind="ExternalInput").ap()
    # out (f32, straight from psum): wide-inst m = (quad, half, g);
    # col = m*16 + (j*4+c)*2 + {0:mean, 1:logvar}
    zout = nc.dram_tensor("z", [128, NM * 16], F32,
                          kind="ExternalOutput").ap()
    if debug:
        dbg_bank = nc.dram_tensor("dbg_bank", [128, 2, SC], F32,
                                  kind="ExternalOutput").ap()
        dbg_prod = nc.dram_tensor("dbg_prod", [128, 2, SC], F16,
                                  kind="ExternalOutput").ap()
        dbg_h = nc.dram_tensor("dbg_h", [128, SC], F16,
                               kind="ExternalOutput").ap()

    from contextlib import ExitStack
    with tile.TileContext(nc) as tc, ExitStack() as st:
        cp = st.enter_context(tc.tile_pool(name="const", bufs=1))
        # warm the activation table before the DMAs so the one-time
        # LoadActFuncSet doesn't gate the first real relu
        warm = cp.tile([128, 1], F16, tag="warm")
        nc.vector.memset(warm[:], 0.0)
        nc.scalar.activation(warm[:], warm[:],
                             mybir.ActivationFunctionType.Relu,
                             bias=0.0, scale=1.0)
        w1t = cp.tile([128, NPAIR, 2, HID], FP8, tag="w1")
        nc.sync.dma_start(w1t[:], w1)
        w1_sb = [w1t[:, p] for p in range(NPAIR)]
        b1_sb = cp.tile([GS, G], F32, tag="b1")
        w2_sb = cp.tile([GS, G, HID], F16, tag="w2")
        sel2_sb = cp.tile([128, 2], F16, tag="sel2")

        def _load_small_consts():
            # emitted after the first x/oh DMAs: b1 is needed by the first
            # relu (~7us in), w2 by the first mm2, sel2 by the first zred
            nc.sync.dma_start(b1_sb[:], b1.rearrange("g k -> k g"))
            nc.sync.dma_start(w2_sb[:], w2.rearrange("g k m -> k g m"))
            nc.sync.dma_start(sel2_sb[:], sel2.rearrange("f k -> k f"))

        xpool = st.enter_context(tc.tile_pool(name="xp", bufs=2))
        ohpool = st.enter_context(tc.tile_pool(name="ohp", bufs=2))
        hpsum = st.enter_context(tc.tile_pool(name="hps", bufs=3, space="PSUM"))
        hpool = st.enter_context(tc.tile_pool(name="hsb", bufs=6))
        bkpsum = st.enter_context(tc.tile_pool(name="bkps", bufs=2, space="PSUM"))
        ppool = st.enter_context(tc.tile_pool(name="prod", bufs=3))
        zpool = st.enter_context(tc.tile_pool(name="zp", bufs=1, space="PSUM"))
        zsbp = st.enter_context(tc.tile_pool(name="zsb", bufs=1))

        zt = zpool.tile([128, 512], F32, tag="z")   # 32 wide-insts per fill
        zsb = zsbp.tile([128, NM * 16], F32, tag="zstage")

        # wide-instance m = (quad, half, g): one group x 1024 batch rows.
        # narrow step n = 2m + j (j = sc-half within the wide instance).
        # software pipeline: mm1(n) | relu(n-2) | mm2(m-2) | had(m-3) | zred(m-4)
        hp_t, hsb_t, bank_t, prod_t = {}, {}, {}, {}
        xg_t, ohq_t = {}, {}
        NN = NM * 2

        def ninfo(n):
            m, j = divmod(n, 2)
            qh, g = divmod(m, G)
            quad, half = divmod(qh, 2)
            return m, j, quad, half, g

        def st_mm1(n):
            m, j, quad, half, g = ninfo(n)
            pair, i = divmod(g, 2)
            so = (half * 2 + j) * SC
            hp = hpsum.tile([HID, SC], F32, name="hp", tag="hpsum")
            nc.tensor.matmul(
                hp[:], w1_sb[pair][64 * i:64 * i + 64],
                xg_t[quad][pair][64 * i:64 * i + 64, :, so:so + SC],
                start=True, stop=True,
                perf_mode=mybir.MatmulPerfMode.DoubleRow,
                tile_position=(64 * i, 0))
            hp_t[n] = hp

        def st_relu(n):
            m, j, quad, half, g = ninfo(n)
            hp = hp_t.pop(n)
            hsb = hpool.tile([HID, SC], F16, name="hsb", tag="h")
            if relu_pat[n] == 'D':
                nc.vector.tensor_scalar(
                    hsb[:], hp[:], b1_sb[:, g:g + 1], 0.0,
                    mybir.AluOpType.add, mybir.AluOpType.max)
            else:
                nc.scalar.activation(
                    hsb[:], hp[:], mybir.ActivationFunctionType.Relu,
                    bias=b1_sb[:, g:g + 1], scale=1.0)
            if debug and n == 0:
                nc.sync.dma_start(dbg_h[:], hsb[:])
            hsb_t[n] = hsb

        def st_mm2(m):
            _, _, quad, half, g = ninfo(2 * m)
            bank = bkpsum.tile([128, 2, SC], F32, name="bank", tag="bank")
            for j in range(2):
                nc.tensor.matmul(bank[:, j], w2_sb[:, g],
                                 hsb_t.pop(2 * m + j)[:],
                                 start=True, stop=True)
            bank_t[m] = bank

        def st_had(m):
            _, _, quad, half, g = ninfo(2 * m)
            bank = bank_t.pop(m)
            prod = ppool.tile([128, 2, SC], F16, name="prod", tag="prod")
            oht = ohq_t[quad][g][:, half]            # [128, 2, SC]
            if debug and m == 0:
                bsb = ppool.tile([128, 2, SC], F32, name="bsb", tag="bdbg")
                nc.vector.tensor_copy(bsb[:], bank[:])
                nc.sync.dma_start(dbg_bank[:], bsb[:])
            nc.vector.tensor_tensor(
                prod[:], bank[:], oht, mybir.AluOpType.mult)
            if debug and m == 0:
                nc.sync.dma_start(dbg_prod[:], prod[:])
            prod_t[m] = prod

        def st_zred(m):
            prod = prod_t.pop(m)
            zoff = (m % 32) * 16
            for c8 in range(8):
                j, c = divmod(c8, 4)
                nc.tensor.matmul(
                    zt[:, zoff + 2 * c8: zoff + 2 * c8 + 2],
                    prod[:, j, 128 * c:128 * c + 128],
                    sel2_sb[:], start=True, stop=True,
                    skip_group_check=True)
            if m % 16 == 15:
                k = (m // 16) % 2
                lo_, hi_ = (m - 15) * 16, (m + 1) * 16
                nc.vector.tensor_copy(zsb[:, lo_:hi_],
                                      zt[:, k * 256:(k + 1) * 256])
                nc.sync.dma_start(zout[:, lo_:hi_], zsb[:, lo_:hi_])

        for n in range(NN + 8):
            if n < NN:
                quad = n // (NN // nquad)
                if n % (NN // nquad) == 0:
                    xg = [xpool.tile([128, 2, QW], FP8, name=f"xg{p}",
                                     tag=f"xg{p}") for p in range(NPAIR)]
                    ohq = [ohpool.tile([128, 2, 2, SC], mybir.dt.int8,
                                       name=f"oh{g}", tag=f"oh{g}")
                           for g in range(G)]
                    # interleave so the pipeline fills asap: x for pair p
                    # arrives just before its mm1s, oh soon after
                    for p in range(NPAIR):
                        if quad == 0:
                            # halves: the first mm1 only needs cols 0:512,
                            # so don't gate it on the full-quad transfer
                            hw_ = QW // 2
                            nc.sync.dma_start(xg[p][:, :, :hw_],
                                              xq[quad, p, :, :, :hw_])
                            nc.sync.dma_start(xg[p][:, :, hw_:],
                                              xq[quad, p, :, :, hw_:])
                        else:
                            nc.sync.dma_start(xg[p][:], xq[quad, p])
                        if quad == 0 and p == 0:
                            _load_small_consts()
                        nc.sync.dma_start(ohq[2 * p][:], oh[quad, 2 * p])
                        nc.sync.dma_start(ohq[2 * p + 1][:],
                                          oh[quad, 2 * p + 1])
                    xg_t[quad] = xg
                    ohq_t[quad] = ohq
                st_mm1(n)
            if 2 <= n < NN + 2:
                st_relu(n - 2)
            if n % 2 == 1:
                m = (n - 1) // 2
                if 2 <= m < NM + 2:
                    st_mm2(m - 2)
                if 3 <= m < NM + 3:
                    st_had(m - 3)
                if 4 <= m < NM + 4:
                    st_zred(m - 4)

    nc.compile()
    return nc


# ---------------------------------------------------------------- host side --

def _prep_host(X, eps, W1, b1, W2, b2, indices, nsc=BPC // SC, ncores=NCORES):
    B = nsc * SC
    nquad = nsc // QUAD
    Xp = np.ascontiguousarray(X[:, GROUP_IDX.reshape(-1)]).reshape(BATCH, G, GS)
    Xp8 = Xp.astype(NP_FP8)
    # W1 DoubleRow pack: (128, pair, 2, hid); partition i*64+p = group 2*pair+i
    # k-slot (p, t) = gs index t*64+p, matching the X pack
    W1r = W1.astype(NP_FP8).reshape(G, 2, 64, HID).transpose(0, 2, 1, 3)
    w1dr = np.ascontiguousarray(
        W1r.reshape(NPAIR, 128, 2, HID).transpose(1, 0, 2, 3))
    w2h = W2.astype(np.float16)
    b1f = b1.astype(np.float32)
    sel2 = np.zeros((2, 128), np.float16)
    sel2[0, :64] = 1.0
    sel2[1, 64:] = 1.0

    in_maps = []
    for core in range(ncores):
        lo = core * B
        Xc = Xp8[lo:lo + B]                          # (B, g, 128)
        Xt = Xc.reshape(B, G, 2, 64).transpose(1, 3, 2, 0)   # (g,64,2,B)
        xpair = Xt.reshape(NPAIR, 128, 2, B)
        xqc = np.ascontiguousarray(
            xpair.reshape(NPAIR, 128, 2, nquad, QW).transpose(3, 0, 1, 2, 4))
        idxc = indices[:, lo:lo + B]                 # (G, B)
        ohm = (idxc[:, None, :] == np.arange(LAT)[None, :, None])  # (G,64,B)
        ohdup = np.concatenate([ohm, ohm], axis=1).astype(np.int8)  # (G,128,B)
        # -> [nquad, g, 128, QW]
        ohc = np.ascontiguousarray(
            ohdup.reshape(G, 128, nquad, QW).transpose(2, 0, 1, 3))
        in_maps.append({
            "xq": xqc, "oh": ohc, "w1": w1dr, "w2": w2h, "b1": b1f,
            "sel2": sel2,
        })
    return in_maps


def _unscramble(zdev, nsc):
    """zdev: (128, nwide*16) f16 -> zM, zL each (G, nsc*SC) f32.

    wide-inst m = (quad, half, g); col = m*16 + (j*4+c)*2 + q;
    batch b = (quad*4 + half*2 + j)*512 + c*128 + p.
    """
    B = nsc * SC
    nquad = nsc // QUAD
    zr = zdev.astype(np.float32).reshape(128, nquad, 2, G, 2, 4, 2)
    # [p, quad, half, g, j, c, q] -> [g, quad, half, j, c, p]
    zM = zr[..., 0].transpose(3, 1, 2, 4, 5, 0).reshape(G, B)
    zL = zr[..., 1].transpose(3, 1, 2, 4, 5, 0).reshape(G, B)
    return zM, zL


def _finish_host(zM, zL, b2, eps_c, idxc):
    b2m_sel = np.take_along_axis(b2[:, :LAT], idxc, axis=1)
    b2v_sel = np.take_along_axis(b2[:, LAT:], idxc, axis=1)
    return zM + b2m_sel + eps_c * np.exp(0.5 * (zL + b2v_sel))


_NC_CACHE = {}


def kernel(X, eps, W1, b1, W2, b2, indices):
    nsc = BPC // SC
    key = (nsc, NCORES)
    if key not in _NC_CACHE:
        _NC_CACHE[key] = build_program(nsc, NCORES)
    nc = _NC_CACHE[key]
    in_maps = _prep_host(X, eps, W1, b1, W2, b2, indices)
    res = bass_utils.run_bass_kernel_spmd(nc, in_maps,
                                          core_ids=list(range(NCORES)))
    z = np.zeros((G, BATCH), np.float32)
    B = nsc * SC
    for core in range(NCORES):
        lo = core * B
        zM, zL = _unscramble(res.results[core]["z"], nsc)
        z[:, lo:lo + B] = _finish_host(zM, zL, b2, eps[:, lo:lo + B],
                                       indices[:, lo:lo + B])
    return z.astype(np.float32)


# revision 4
# speedup vs baseline: 1.0093x; 1.0044x over previous
"""EnVAE sampling kernel v2 for 8x TRN2 NeuronCores.

Math (per group g, batch element b):
  Xg = X[:, g::8]                                      # (b, 128)
  h  = relu(Xg @ W1[g] + b1[g])                        # (b, 128)
  out= h @ W2[g] + b2[g]; means=out[:, :64]; lv=out[:, 64:]
  z  = means[b, idx] + eps * exp(0.5 * lv[b, idx])

Device (batch-sharded 8 ways):
  mm1  fp8e4 DoubleRow:  hp = W1g^T Xg         (PE, 0.5 cyc/row)
  relu ACT/GPS:          h = relu(hp + b1)     -> fp16
  mm2  fp16 combined:    bank = W2g^T h        # [128 = 64 mean | 64 logvar, b]
  had  DVE/GPS (pair-wide): prod = bank * ohdup
  zred PE:               z[b, 0] = sum_top64 prod, z[b, 1] = sum_bot64 prod
Host finishes: z = zM + b2m[idx] + eps * exp(0.5*(zL + b2v[idx]))
"""

import numpy as np
import ml_dtypes

import concourse.bass as bass
import concourse.bacc as bacc
import concourse.mybir as mybir
from concourse import tile
from concourse import bass_utils

OBS = 1024
LAT = 64
G = 8
GS = 128
HID = 128
BATCH = 65536
NCORES = 8
BPC = BATCH // NCORES        # 8192 batch rows per core
SC = 512                     # batch rows per chunk instance
NPAIR = G // 2
QUAD = 4                     # sc-chunks per quad (DMA granule)
QW = QUAD * SC               # 2048
F16 = mybir.dt.float16
F32 = mybir.dt.float32
FP8 = mybir.dt.float8e4
NP_FP8 = ml_dtypes.float8_e4m3fn

GROUP_IDX = np.stack([np.arange(n, OBS, G) for n in range(G)])  # (g, gs)


def _mk_pattern(total, n_gps):
    pat = ['x'] * total
    if n_gps > 0:
        for k in range(n_gps):
            pat[(k * total) // n_gps] = 'G'
    return pat


def build_program(nsc: int, num_devices: int = NCORES, relu_dve=0, debug=False):
    """Per-core program for nsc chunks of SC batch rows."""
    B = nsc * SC
    nquad = nsc // QUAD
    NM = (nsc // 2) * G                 # 64 wide-instances (g, 2*sc)
    NREL = NM * 2                       # 128 narrow relu ops
    nc = bacc.Bacc("TRN2", target_bir_lowering=False, debug=False,
                   num_devices=num_devices)

    relu_pat = ['x'] * NREL                  # 'x' -> ACT, 'D' -> DVE
    for k in range(relu_dve):
        relu_pat[(k * NREL) // relu_dve] = 'D'

    # DRAM inputs (per-core shard)
    xq = nc.dram_tensor("xq", [nquad, NPAIR, 128, 2, QW], FP8,
                        kind="ExternalInput").ap()
    # onehot (dup'd to 128 rows), per group: [nquad, g, 128, QW]
    oh = nc.dram_tensor("oh", [nquad, G, 128, QW], mybir.dt.int8,
                        kind="ExternalInput").ap()
    w1 = nc.dram_tensor("w1", [128, NPAIR, 2, HID], FP8,
                        kind="ExternalInput").ap()
    w2 = nc.dram_tensor("w2", [G, GS, HID], F16, kind="ExternalInput").ap()
    b1 = nc.dram_tensor("b1", [G, GS], F32, kind="ExternalInput").ap()
    sel2 = nc.dram_tensor("sel2", [2, 128], F16, kind="ExternalInput").ap()
    # out (f32, straight from psum): wide-inst m = (quad, half, g);
    # col = m*16 + (j*4+c)*2 + {0:mean, 1:logvar}
    zout = nc.dram_tensor("z", [128, NM * 16], F32,
                          kind="ExternalOutput").ap()
    if debug:
        dbg_bank = nc.dram_tensor("dbg_bank", [128, 2, SC], F32,
                                  kind="ExternalOutput").ap()
        dbg_prod = nc.dram_tensor("dbg_prod", [128, 2, SC], F16,
                                  kind="ExternalOutput").ap()
        dbg_h = nc.dram_tensor("dbg_h", [128, SC], F16,
                               kind="ExternalOutput").ap()

    from contextlib import ExitStack
    with tile.TileContext(nc) as tc, ExitStack() as st:
        cp = st.enter_context(tc.tile_pool(name="const", bufs=1))
        # warm the activation table before the DMAs so the one-time
        # LoadActFuncSet doesn't gate the first real relu
        warm = cp.tile([128, 1], F16, tag="warm")
        nc.vector.memset(warm[:], 0.0)
        nc.scalar.activation(warm[:], warm[:],
                             mybir.ActivationFunctionType.Relu,
                             bias=0.0, scale=1.0)
        w1t = cp.tile([128, NPAIR, 2, HID], FP8, tag="w1")
        nc.sync.dma_start(w1t[:], w1)
        w1_sb = [w1t[:, p] for p in range(NPAIR)]
        b1_sb = cp.tile([GS, G], F32, tag="b1")
        w2_sb = cp.tile([GS, G, HID], F16, tag="w2")
        sel2_sb = cp.tile([128, 2], F16, tag="sel2")

        def _load_small_consts():
            # emitted after the first x/oh DMAs: b1 is needed by the first
            # relu (~7us in), w2 by the first mm2, sel2 by the first zred
            nc.sync.dma_start(b1_sb[:], b1.rearrange("g k -> k g"))
            nc.sync.dma_start(w2_sb[:], w2.rearrange("g k m -> k g m"))
            nc.sync.dma_start(sel2_sb[:], sel2.rearrange("f k -> k f"))

        xpool = st.enter_context(tc.tile_pool(name="xp", bufs=2))
        ohpool = st.enter_context(tc.tile_pool(name="ohp", bufs=2))
        hpsum = st.enter_context(tc.tile_pool(name="hps", bufs=3, space="PSUM"))
        hpool = st.enter_context(tc.tile_pool(name="hsb", bufs=6))
        bkpsum = st.enter_context(tc.tile_pool(name="bkps", bufs=2, space="PSUM"))
        ppool = st.enter_context(tc.tile_pool(name="prod", bufs=3))
        zpool = st.enter_context(tc.tile_pool(name="zp", bufs=1, space="PSUM"))
        zsbp = st.enter_context(tc.tile_pool(name="zsb", bufs=1))

        zt = zpool.tile([128, 512], F32, tag="z")   # 32 wide-insts per fill
        zsb = zsbp.tile([128, NM * 16], F32, tag="zstage")

        # wide-instance m = (quad, half, g): one group x 1024 batch rows.
        # narrow step n = 2m + j (j = sc-half within the wide instance).
        # software pipeline: mm1(n) | relu(n-2) | mm2(m-2) | had(m-3) | zred(m-4)
        hp_t, hsb_t, bank_t, prod_t = {}, {}, {}, {}
        xg_t, ohq_t = {}, {}
        NN = NM * 2

        def ninfo(n):
            m, j = divmod(n, 2)
            qh, g = divmod(m, G)
            quad, half = divmod(qh, 2)
            return m, j, quad, half, g

        def st_mm1(n):
            m, j, quad, half, g = ninfo(n)
            pair, i = divmod(g, 2)
            so = (half * 2 + j) * SC
            hp = hpsum.tile([HID, SC], F32, name="hp", tag="hpsum")
            nc.tensor.matmul(
                hp[:], w1_sb[pair][64 * i:64 * i + 64],
                xg_t[quad][pair][64 * i:64 * i + 64, :, so:so + SC],
                start=True, stop=True,
                perf_mode=mybir.MatmulPerfMode.DoubleRow,
                tile_position=(64 * i, 0))
            hp_t[n] = hp

        def st_relu(n):
            m, j, quad, half, g = ninfo(n)
            hp = hp_t.pop(n)
            hsb = hpool.tile([HID, SC], F16, name="hsb", tag="h")
            if relu_pat[n] == 'D':
                nc.vector.tensor_scalar(
                    hsb[:], hp[:], b1_sb[:, g:g + 1], 0.0,
                    mybir.AluOpType.add, mybir.AluOpType.max)
            else:
                nc.scalar.activation(
                    hsb[:], hp[:], mybir.ActivationFunctionType.Relu,
                    bias=b1_sb[:, g:g + 1], scale=1.0)
            if debug and n == 0:
                nc.sync.dma_start(dbg_h[:], hsb[:])
            hsb_t[n] = hsb

        def st_mm2(m):
            _, _, quad, half, g = ninfo(2 * m)
            bank = bkpsum.tile([128, 2, SC], F32, name="bank", tag="bank")
            for j in range(2):
                nc.tensor.matmul(bank[:, j], w2_sb[:, g],
                                 hsb_t.pop(2 * m + j)[:],
                                 start=True, stop=True)
            bank_t[m] = bank

        def st_had(m):
            _, _, quad, half, g = ninfo(2 * m)
            bank = bank_t.pop(m)
            prod = ppool.tile([128, 2, SC], F16, name="prod", tag="prod")
            oht = ohq_t[quad][g][:, half]            # [128, 2, SC]
            if debug and m == 0:
                bsb = ppool.tile([128, 2, SC], F32, name="bsb", tag="bdbg")
                nc.vector.tensor_copy(bsb[:], bank[:])
                nc.sync.dma_start(dbg_bank[:], bsb[:])
            nc.vector.tensor_tensor(
                prod[:], bank[:], oht, mybir.AluOpType.mult)
            if debug and m == 0:
                nc.sync.dma_start(dbg_prod[:], prod[:])
            prod_t[m] = prod

        def st_zred(m):
            prod = prod_t.pop(m)
            zoff = (m % 32) * 16
            for c8 in range(8):
                j, c = divmod(c8, 4)
                nc.tensor.matmul(
                    zt[:, zoff + 2 * c8: zoff + 2 * c8 + 2],
                    prod[:, j, 128 * c:128 * c + 128],
                    sel2_sb[:], start=True, stop=True,
                    skip_group_check=True)
            if m % 8 == 7:
                ko = (m % 32 - 7) * 16
                lo_, hi_ = (m - 7) * 16, (m + 1) * 16
                nc.vector.tensor_copy(zsb[:, lo_:hi_],
                                      zt[:, ko:ko + 128])
                nc.sync.dma_start(zout[:, lo_:hi_], zsb[:, lo_:hi_])

        for n in range(NN + 8):
            if n < NN:
                quad = n // (NN // nquad)
                if n % (NN // nquad) == 0:
                    xg = [xpool.tile([128, 2, QW], FP8, name=f"xg{p}",
                                     tag=f"xg{p}") for p in range(NPAIR)]
                    ohq = [ohpool.tile([128, 2, 2, SC], mybir.dt.int8,
                                       name=f"oh{g}", tag=f"oh{g}")
                           for g in range(G)]
                    # interleave so the pipeline fills asap: x for pair p
                    # arrives just before its mm1s, oh soon after
                    for p in range(NPAIR):
                        if quad == 0:
                            # halves: the first mm1 only needs cols 0:512,
                            # so don't gate it on the full-quad transfer
                            hw_ = QW // 2
                            nc.sync.dma_start(xg[p][:, :, :hw_],
                                              xq[quad, p, :, :, :hw_])
                            nc.sync.dma_start(xg[p][:, :, hw_:],
                                              xq[quad, p, :, :, hw_:])
                        else:
                            nc.sync.dma_start(xg[p][:], xq[quad, p])
                        if quad == 0 and p == 0:
                            _load_small_consts()
                        nc.sync.dma_start(ohq[2 * p][:], oh[quad, 2 * p])
                        nc.sync.dma_start(ohq[2 * p + 1][:],
                                          oh[quad, 2 * p + 1])
                    xg_t[quad] = xg
                    ohq_t[quad] = ohq
                st_mm1(n)
            if 2 <= n < NN + 2:
                st_relu(n - 2)
            if n % 2 == 1:
                m = (n - 1) // 2
                if 2 <= m < NM + 2:
                    st_mm2(m - 2)
                if 3 <= m < NM + 3:
                    st_had(m - 3)
                if 4 <= m < NM + 4:
                    st_zred(m - 4)

    nc.compile()
    return nc


# ---------------------------------------------------------------- host side --

def _prep_host(X, eps, W1, b1, W2, b2, indices, nsc=BPC // SC, ncores=NCORES):
    B = nsc * SC
    nquad = nsc // QUAD
    Xp = np.ascontiguousarray(X[:, GROUP_IDX.reshape(-1)]).reshape(BATCH, G, GS)
    Xp8 = Xp.astype(NP_FP8)
    # W1 DoubleRow pack: (128, pair, 2, hid); partition i*64+p = group 2*pair+i
    # k-slot (p, t) = gs index t*64+p, matching the X pack
    W1r = W1.astype(NP_FP8).reshape(G, 2, 64, HID).transpose(0, 2, 1, 3)
    w1dr = np.ascontiguousarray(
        W1r.reshape(NPAIR, 128, 2, HID).transpose(1, 0, 2, 3))
    w2h = W2.astype(np.float16)
    b1f = b1.astype(np.float32)
    sel2 = np.zeros((2, 128), np.float16)
    sel2[0, :64] = 1.0
    sel2[1, 64:] = 1.0

    in_maps = []
    for core in range(ncores):
        lo = core * B
        Xc = Xp8[lo:lo + B]                          # (B, g, 128)
        Xt = Xc.reshape(B, G, 2, 64).transpose(1, 3, 2, 0)   # (g,64,2,B)
        xpair = Xt.reshape(NPAIR, 128, 2, B)
        xqc = np.ascontiguousarray(
            xpair.reshape(NPAIR, 128, 2, nquad, QW).transpose(3, 0, 1, 2, 4))
        idxc = indices[:, lo:lo + B]                 # (G, B)
        ohm = (idxc[:, None, :] == np.arange(LAT)[None, :, None])  # (G,64,B)
        ohdup = np.concatenate([ohm, ohm], axis=1).astype(np.int8)  # (G,128,B)
        # -> [nquad, g, 128, QW]
        ohc = np.ascontiguousarray(
            ohdup.reshape(G, 128, nquad, QW).transpose(2, 0, 1, 3))
        in_maps.append({
            "xq": xqc, "oh": ohc, "w1": w1dr, "w2": w2h, "b1": b1f,
            "sel2": sel2,
        })
    return in_maps


def _unscramble(zdev, nsc):
    """zdev: (128, nwide*16) f16 -> zM, zL each (G, nsc*SC) f32.

    wide-inst m = (quad, half, g); col = m*16 + (j*4+c)*2 + q;
    batch b = (quad*4 + half*2 + j)*512 + c*128 + p.
    """
    B = nsc * SC
    nquad = nsc // QUAD
    zr = zdev.astype(np.float32).reshape(128, nquad, 2, G, 2, 4, 2)
    # [p, quad, half, g, j, c, q] -> [g, quad, half, j, c, p]
    zM = zr[..., 0].transpose(3, 1, 2, 4, 5, 0).reshape(G, B)
    zL = zr[..., 1].transpose(3, 1, 2, 4, 5, 0).reshape(G, B)
    return zM, zL


def _finish_host(zM, zL, b2, eps_c, idxc):
    b2m_sel = np.take_along_axis(b2[:, :LAT], idxc, axis=1)
    b2v_sel = np.take_along_axis(b2[:, LAT:], idxc, axis=1)
    return zM + b2m_sel + eps_c * np.exp(0.5 * (zL + b2v_sel))


_NC_CACHE = {}


def kernel(X, eps, W1, b1, W2, b2, indices):
    nsc = BPC // SC
    key = (nsc, NCORES)
    if key not in _NC_CACHE:
        _NC_CACHE[key] = build_program(nsc, NCORES)
    nc = _NC_CACHE[key]
    in_maps = _prep_host(X, eps, W1, b1, W2, b2, indices)
    res = bass_utils.run_bass_kernel_spmd(nc, in_maps,
                                          core_ids=list(range(NCORES)))
    z = np.zeros((G, BATCH), np.float32)
    B = nsc * SC
    for core in range(NCORES):
        lo = core * B
        zM, zL = _unscramble(res.results[core]["z"], nsc)
        z[:, lo:lo + B] = _finish_host(zM, zL, b2, eps[:, lo:lo + B],
                                       indices[:, lo:lo + B])
    return z.astype(np.float32)
